# revision 1
# baseline (speedup 1.0000x reference)
"""2-layer GraphSAGE (mean aggregation) on 8 Trainium2 NeuronCores.

Strategy (dst-sharded graph parallel):
- 100000 nodes padded to 100352 = 8 x 12544 (12500 real + 44 pad per core).
- Core k owns dst nodes [k*12500, (k+1)*12500); edges grouped by dst core.
- Per core, dsts are processed in 98 blocks of 128. Edges of a block are
  sorted by src chunk (4 chunks of 32768 global rows, int16-indexable) and
  gathered compactly with dma_gather (<=512 rows/call, single_packet) from a
  replicated global feature table in device DRAM into SBUF staging.
- Per 128-position staging tile, a one-hot indicator [128 pos, 128 dst]
  (built on DVE via tensor_scalar is_equal against an iota row) is matmul'ed
  on PE with the staged rows, accumulating per-dst sums directly in PSUM.
  Padding / skipped positions carry a sentinel seg id -> zero column.
- mean = PSUM * (1/cnt) via ACT copy with per-partition scale; layer output
  = mean @ W_l + x @ W_r + b (+ReLU for layer 1) with PE transposes and a
  rank-1 matmul for the bias.
- h1 shards are AllGather'ed (64 wide), then locally expanded into a
  512B-row padded table [100352, 128] (upper half garbage, never read) so
  layer-2 gathers also run at the fast 512B descriptor size.
- Both layers share the same edge structure: one idx/seg tensor pair.
"""
import sys
sys.path.insert(0, "/opt/trn_rl_repo")
import numpy as np

import concourse.bass as bass
import concourse.bacc as bacc
import concourse.mybir as mybir
import concourse.tile as tile
from concourse.bass_utils import run_bass_kernel_spmd

N_NODES = 100000
N_EDGES = 1600000
F_IN = 128
F_OUT = 64
P = 8                 # cores
NREAL = 12500         # real dsts per core
NL = 12544            # padded dsts per core
BLK = 112             # dsts per block (112*112 = 12544); keeps each
                      # (block, chunk) edge segment under one 512-row call
NB = NL // BLK        # 112 blocks
NG = P * NL           # 100352 padded global rows
CHUNK = 32768
NCHUNK = (NG + CHUNK - 1) // CHUNK   # 4
MAX_CALL = 512        # rows per dma_gather call (single_packet limit)
SENT = 999.0          # sentinel seg id


def _wrap16(flat_idx):
    """dma_gather index layout: position j -> [j%16, j//16], replicated x8."""
    w = flat_idx.reshape(-1, 16).T.copy()
    return np.tile(w, (8, 1))


def _preprocess(edge_index):
    """Host-side graph layout. Returns per-core dicts + global structure."""
    src = np.asarray(edge_index[0], dtype=np.int64)
    dst = np.asarray(edge_index[1], dtype=np.int64)
    dcore = dst // NREAL
    dslot = dst - dcore * NREAL          # 0..12499
    # global padded row id of a node
    score = src // NREAL
    g_src = score * NL + (src - score * NREAL)

    # per (core, block, chunk) edge lists
    blk = dslot // BLK
    m_in_blk = dslot % BLK
    chunk = g_src // CHUNK

    # counts per core for mean
    rcnt_all = []
    for k in range(P):
        sel = dcore == k
        cnt = np.bincount(dslot[sel], minlength=NL).astype(np.float32)
        rcnt = 1.0 / np.maximum(cnt, 1.0)
        r = np.ones((128, NB), np.float32)
        r[:BLK, :] = rcnt.reshape(NB, BLK).T
        rcnt_all.append(r)   # [128, NB], rows >= BLK unused

    # per (core, b, c): sorted edge arrays
    ncols_bc = np.zeros((P, NB, NCHUNK), dtype=np.int64)
    per_core = [[[None] * NCHUNK for _ in range(NB)] for _ in range(P)]
    order = np.lexsort((g_src, chunk, blk, dcore))
    src_o, g_o, dc_o, bl_o, ch_o, m_o = (
        src[order], g_src[order], dcore[order], blk[order], chunk[order], m_in_blk[order])
    # group boundaries
    key = ((dc_o * NB + bl_o) * NCHUNK + ch_o)
    bounds = np.flatnonzero(np.diff(key)) + 1
    starts = np.concatenate([[0], bounds])
    ends = np.concatenate([bounds, [len(key)]])
    for s, e in zip(starts, ends):
        k, b, c = int(dc_o[s]), int(bl_o[s]), int(ch_o[s])
        n = e - s
        ncols_bc[k, b, c] = (n + 127) // 128
        per_core[k][b][c] = (g_o[s:e] - c * CHUNK, m_o[s:e])

    # uniform structure: max cols over cores
    ncols_u = ncols_bc.max(axis=0)       # [NB, NCHUNK]

    # build call structure (global) and per-core idx/seg streams
    calls = []     # (b, c, start_col_in_block, ncols_piece, idx_col_off)
    tiles_b = []   # tiles per block
    idx_off = 0
    seg_off = 0
    block_meta = []
    for b in range(NB):
        pos_col = 0
        for c in range(NCHUNK):
            nc_cols = int(ncols_u[b, c])
            done = 0
            while done < nc_cols:
                piece = min(nc_cols - done, MAX_CALL // 128)
                calls.append((b, c, pos_col + done, piece, idx_off))
                idx_off += piece * 8          # idx cols (16 pos each)
                done += piece
            pos_col += nc_cols
        tiles_b.append(pos_col)
        block_meta.append(seg_off)
        seg_off += pos_col
    total_cols = seg_off                      # total 128-position tiles
    IDX_COLS = idx_off

    idx_cores = []
    seg_cores = []
    for k in range(P):
        # dummy positions gather row 0 of their chunk (valid address, cheap);
        # the sentinel seg id zeroes their contribution. All-negative calls
        # hang the SWDGE firmware, so never emit skipped indices.
        idx_flat = np.full(total_cols * 128, 0, dtype=np.int64)
        seg_flat = np.full(total_cols * 128, SENT, dtype=np.float32)
        for b in range(NB):
            pos0 = block_meta[b] * 128
            pos_col = 0
            for c in range(NCHUNK):
                ent = per_core[k][b][c]
                if ent is not None:
                    g, m = ent
                    n = len(g)
                    st = pos0 + pos_col * 128
                    idx_flat[st:st + n] = g
                    seg_flat[st:st + n] = m
                pos_col += int(ncols_u[b, c])
        # idx wrapped per call piece: wrapping is uniform over positions, and
        # call slices are 16-position-aligned, so one global wrap works.
        idx_cores.append(_wrap16(idx_flat.astype(np.int16)))      # [128, total_cols*8]
        seg_cores.append(seg_flat.reshape(total_cols, 128).T.copy())  # [128, total_cols]

    return dict(calls=calls, tiles_b=tiles_b, block_meta=block_meta,
                total_cols=total_cols, IDX_COLS=IDX_COLS,
                idx_cores=idx_cores, seg_cores=seg_cores, rcnt=rcnt_all)


def _build(meta, mode="full", nb_limit=None, rep=1):
    calls = meta["calls"]
    tiles_b = meta["tiles_b"]
    block_meta = meta["block_meta"]
    TC = meta["total_cols"]
    IDX_COLS = meta["IDX_COLS"]
    T_max = max(tiles_b)

    nc = bacc.Bacc("TRN2", target_bir_lowering=False, debug=False,
                   num_devices=P, num_swdge_queues=4)
    dt = mybir.dt
    x_full = nc.dram_tensor("x_full", [NG, F_IN], dt.float32, kind="ExternalInput")
    x_shard = nc.dram_tensor("x_shard", [NL, F_IN], dt.float32, kind="ExternalInput")
    idx_d = nc.dram_tensor("idx", [128, IDX_COLS], dt.int16, kind="ExternalInput")
    seg_d = nc.dram_tensor("seg", [128, TC], dt.float32, kind="ExternalInput")
    rcnt_d = nc.dram_tensor("rcnt", [128, NB], dt.float32, kind="ExternalInput")
    iota_d = nc.dram_tensor("iota", [128, BLK], dt.float32, kind="ExternalInput")
    wl1_d = nc.dram_tensor("W_l1", [F_IN, F_OUT], dt.float32, kind="ExternalInput")
    wr1_d = nc.dram_tensor("W_r1", [F_IN, F_OUT], dt.float32, kind="ExternalInput")
    b1_d = nc.dram_tensor("b1", [1, F_OUT], dt.float32, kind="ExternalInput")
    wl2_d = nc.dram_tensor("W_l2", [F_OUT, F_OUT], dt.float32, kind="ExternalInput")
    wr2_d = nc.dram_tensor("W_r2", [F_OUT, F_OUT], dt.float32, kind="ExternalInput")
    b2_d = nc.dram_tensor("b2", [1, F_OUT], dt.float32, kind="ExternalInput")
    out_d = nc.dram_tensor("out", [NL, F_OUT], dt.float32, kind="ExternalOutput")

    # internal DRAM
    h1_shard = nc.dram_tensor("h1_shard", [NL, F_OUT], dt.float32)
    h1_full = nc.dram_tensor("h1_full", [NG, F_OUT], dt.float32, addr_space="Shared")
    h1_pad = nc.dram_tensor("h1_pad", [NG, F_IN], dt.float32)

    ident = mybir.ActivationFunctionType

    with tile.TileContext(nc) as tc:
        with (
            tc.tile_pool(name="const", bufs=1) as constp,
            tc.tile_pool(name="stagep", bufs=4) as stagep,
            tc.tile_pool(name="indp", bufs=6) as indp,
            tc.tile_pool(name="xp", bufs=3) as xp,
            tc.tile_pool(name="op", bufs=3) as op,
            tc.tile_pool(name="ps_acc", bufs=3, space="PSUM") as ps_acc,
            tc.tile_pool(name="ps_t", bufs=3, space="PSUM") as ps_t,
            tc.tile_pool(name="ps_o", bufs=2, space="PSUM") as ps_o,
        ):
            iota_t = constp.tile([128, BLK], dt.float32)
            nc.sync.dma_start(iota_t[:], iota_d[:])
            seg_t = constp.tile([128, TC], dt.float32)
            nc.sync.dma_start(seg_t[:], seg_d[:])
            idx_t = constp.tile([128, IDX_COLS], dt.int16)
            nc.sync.dma_start(idx_t[:], idx_d[:])
            rcnt_t = constp.tile([128, NB], dt.float32)
            nc.sync.dma_start(rcnt_t[:], rcnt_d[:])
            wl1_t = constp.tile([F_IN, F_OUT], dt.float32)
            nc.sync.dma_start(wl1_t[:], wl1_d[:])
            wr1_t = constp.tile([F_IN, F_OUT], dt.float32)
            nc.sync.dma_start(wr1_t[:], wr1_d[:])
            wl2_t = constp.tile([F_OUT, F_OUT], dt.float32)
            nc.sync.dma_start(wl2_t[:], wl2_d[:])
            wr2_t = constp.tile([F_OUT, F_OUT], dt.float32)
            nc.sync.dma_start(wr2_t[:], wr2_d[:])
            b1_t = constp.tile([1, F_OUT], dt.float32)
            nc.sync.dma_start(b1_t[:], b1_d[:])
            b2_t = constp.tile([1, F_OUT], dt.float32)
            nc.sync.dma_start(b2_t[:], b2_d[:])
            ones_t = constp.tile([1, 128], dt.float32)
            nc.vector.memset(ones_t[:], 1.0)
            from concourse.masks import make_identity
            id_t = constp.tile([BLK, BLK], dt.float32)
            make_identity(nc, id_t[:])

            qn = [0]

            def layer(li, table, FW, wl_t, wr_t, bias_t, self_src, out_dram, relu):
                """One SAGE layer. table rows are FW fp32 wide (512B); the
                aggregated features are the first FW_used columns."""
                FW_used = F_IN if li == 1 else F_OUT
                for b in range(NB if nb_limit is None else nb_limit):
                    Tb = tiles_b[b]
                    stage = stagep.tile([128, T_max * F_IN], dt.float32, tag="stage")
                    for (bb, c, start_col, piece, idx_off) in calls:
                        if bb != b:
                            continue
                        nc.gpsimd.dma_gather(
                            out_ap=stage[:, start_col * F_IN:(start_col + piece) * F_IN]
                                .rearrange("p (c f) -> p c f", f=F_IN),
                            in_ap=table[c * CHUNK:min((c + 1) * CHUNK, NG), :],
                            idxs_ap=idx_t[:, idx_off:idx_off + piece * 8],
                            num_idxs=piece * 128, num_idxs_reg=piece * 128,
                            elem_size=F_IN, single_packet=True,
                            queue_num=qn[0] % 4)
                        qn[0] += 1
                    if mode == "l1g":
                        nc.sync.dma_start(out_dram[b * BLK:(b + 1) * BLK, :],
                                          stage[:BLK, :F_OUT])
                        continue
                    soff = block_meta[b]
                    acc = ps_acc.tile([BLK, FW_used], dt.float32, tag="acc")
                    for t in range(Tb):
                        ind = indp.tile([128, BLK], dt.float32, tag="ind")
                        nc.vector.tensor_scalar(
                            out=ind[:], in0=iota_t[:],
                            scalar1=seg_t[:, soff + t:soff + t + 1],
                            scalar2=None, op0=mybir.AluOpType.is_equal)
                        nc.tensor.matmul(
                            acc[:], lhsT=ind[:],
                            rhs=stage[:, t * F_IN:t * F_IN + FW_used],
                            start=(t == 0), stop=(t == Tb - 1))
                    # mean (node-major) with 1/cnt scale
                    mean = op.tile([BLK, FW_used], dt.float32, tag="mean")
                    nc.scalar.activation(out=mean[:], in_=acc[:], func=ident.Copy,
                                         scale=rcnt_t[:BLK, b:b + 1])
                    # transpose mean -> [FW_used, 128]
                    mt_ps = ps_t.tile([FW_used, BLK], dt.float32, tag="tp")
                    nc.tensor.transpose(out=mt_ps[:], in_=mean[:], identity=id_t[:])
                    meanT = op.tile([FW_used, BLK], dt.float32, tag="meanT")
                    nc.scalar.activation(out=meanT[:], in_=mt_ps[:], func=ident.Copy)
                    # self term transpose
                    xb = xp.tile([BLK, FW_used], dt.float32, tag="xb")
                    nc.sync.dma_start(xb[:], self_src[b * BLK:(b + 1) * BLK, :])
                    xt_ps = ps_t.tile([FW_used, BLK], dt.float32, tag="tp")
                    nc.tensor.transpose(out=xt_ps[:], in_=xb[:], identity=id_t[:])
                    xbT = op.tile([FW_used, BLK], dt.float32, tag="xbT")
                    nc.scalar.activation(out=xbT[:], in_=xt_ps[:], func=ident.Copy)
                    # out = mean @ W_l + x @ W_r + b
                    o_ps = ps_o.tile([BLK, F_OUT], dt.float32, tag="ops")
                    nc.tensor.matmul(o_ps[:], lhsT=meanT[:], rhs=wl_t[:],
                                     start=True, stop=False)
                    nc.tensor.matmul(o_ps[:], lhsT=xbT[:], rhs=wr_t[:],
                                     start=False, stop=False)
                    nc.tensor.matmul(o_ps[:], lhsT=ones_t[:1, :BLK], rhs=bias_t[:],
                                     start=False, stop=True)
                    ob = op.tile([BLK, F_OUT], dt.float32, tag="ob")
                    if relu:
                        nc.vector.tensor_scalar_max(ob[:], o_ps[:], 0.0)
                    else:
                        nc.vector.tensor_copy(ob[:], o_ps[:])
                    nc.sync.dma_start(out_dram[b * BLK:(b + 1) * BLK, :], ob[:])

            for _r in range(rep):
              # ---- layer 1 ----
              layer(1, x_full, F_IN, wl1_t, wr1_t, b1_t, x_shard,
                    out_d if mode in ("l1", "l1g") else h1_shard, True)

              if mode not in ("l1", "l1g"):
                  # ---- exchange ----
                  nc.gpsimd.collective_compute(
                      "AllGather", mybir.AluOpType.bypass,
                      replica_groups=[list(range(P))],
                      ins=[h1_shard[:]], outs=[h1_full[:]])
                  if mode == "l1+ag":
                      nc.sync.dma_start(out_d[:], h1_full[:NL, :])
                  else:
                      # expand to 512B rows (upper half garbage, never read);
                      # split rows to stay under the 16-bit per-dim ISA limit
                      EXP = NG // 4
                      for e in range(4):
                          nc.sync.dma_start(h1_pad[e * EXP:(e + 1) * EXP, :F_OUT],
                                            h1_full[e * EXP:(e + 1) * EXP, :])
                      if mode == "l1+ag+expand":
                          nc.sync.dma_start(out_d[:], h1_pad[:NL, :F_OUT])
                      else:
                          # ---- layer 2 ----
                          layer(2, h1_pad, F_IN, wl2_t, wr2_t, b2_t, h1_shard,
                                out_d, False)

    nc.finalize()
    return nc


_CACHE = {}


def kernel(x, edge_index, W_l1, W_r1, b1, W_l2, W_r2, b2, _mode="full", _nb=None):
    x = np.asarray(x, dtype=np.float32)
    meta = _preprocess(np.asarray(edge_index))

    # global padded feature table, replicated
    x_full = np.zeros((NG, F_IN), dtype=np.float32)
    for k in range(P):
        x_full[k * NL:k * NL + NREAL] = x[k * NREAL:(k + 1) * NREAL]

    iota = np.broadcast_to(np.arange(BLK, dtype=np.float32), (128, BLK)).copy()
    in_maps = []
    for k in range(P):
        in_maps.append({
            "x_full": x_full,
            "x_shard": x_full[k * NL:(k + 1) * NL],
            "idx": meta["idx_cores"][k],
            "seg": meta["seg_cores"][k],
            "rcnt": meta["rcnt"][k],
            "iota": iota,
            "W_l1": np.asarray(W_l1, np.float32),
            "W_r1": np.asarray(W_r1, np.float32),
            "b1": np.asarray(b1, np.float32).reshape(1, F_OUT),
            "W_l2": np.asarray(W_l2, np.float32),
            "W_r2": np.asarray(W_r2, np.float32),
            "b2": np.asarray(b2, np.float32).reshape(1, F_OUT),
        })

    nc = _build(meta, mode=_mode, nb_limit=_nb)
    res = run_bass_kernel_spmd(nc, in_maps, core_ids=list(range(P)))
    out = np.concatenate(
        [res.results[k]["out"][:NREAL] for k in range(P)], axis=0)
    return out.astype(np.float32)


if __name__ == "__main__":
    rng = np.random.default_rng(0)
    x = rng.normal(size=(N_NODES, F_IN)).astype(np.float32)
    ei = rng.integers(0, N_NODES, size=(2, N_EDGES)).astype(np.int64)
    wl1 = rng.normal(size=(F_IN, F_OUT)).astype(np.float32) / np.sqrt(F_IN)
    wr1 = rng.normal(size=(F_IN, F_OUT)).astype(np.float32) / np.sqrt(F_IN)
    wl2 = rng.normal(size=(F_OUT, F_OUT)).astype(np.float32) / np.sqrt(F_OUT)
    wr2 = rng.normal(size=(F_OUT, F_OUT)).astype(np.float32) / np.sqrt(F_OUT)
    b1 = np.zeros(F_OUT, np.float32)
    b2 = np.zeros(F_OUT, np.float32)
    out = kernel(x, ei, wl1, wr1, b1, wl2, wr2, b2)
    print("out", out.shape, out.dtype, float(np.abs(out).mean()))



# revision 3
# speedup vs baseline: 1.6780x; 1.6780x over previous
"""2-layer GraphSAGE (mean aggregation) on 8 Trainium2 NeuronCores — v2.

Strategy (dst-sharded, transform-first, bf16 tables):
- 100000 nodes padded to 100352 = 8 x 12544 (12500 real per core). Core k
  owns dst nodes [k*12500, (k+1)*12500), processed as NB=98 blocks of 128.
- Transform-first: layer-l messages are y_l = h @ W_l computed BEFORE
  aggregation (mean commutes with the linear map), so gathers move 64-wide
  transformed features instead of 128-wide raw ones. Tables are stored as
  [100352, 128] bf16 rows (256B — the dma_gather minimum elem size; upper
  64 cols are never read). Each core computes its own shard's table tile
  on PE (bf16 matmuls) and the shards are AllGather'ed.
- Edges grouped by (dst block, src chunk of 25088 rows) — 4 chunks keep
  gather indices int16-addressable. Segments padded to 128-position tiles
  (idx 0 + sentinel seg id -> zero contribution).
- The position stream is ordered chunk-major within GROUPs of blocks so a
  single dma_gather call (one source chunk) spans many blocks: far fewer
  calls -> amortizes the ~1us fixed SWDGE descriptor-generation cost.
- Aggregation per 128-position tile: indicator [128 pos, 128 dst] built on
  DVE in bf16 with ONE fused scalar_tensor_tensor per (block, chunk)
  (iota_rep == broadcast seg), then bf16 matmuls accumulate per-dst sums
  in PSUM (1 cycle/row vs 4 for fp32).
- Block epilogue: out = (acc * 1/cnt) + y_r[block] in one fused DVE op
  (y_r = h @ W_r + b, self term, computed per-shard and kept in SBUF);
  ReLU+bf16-cast for layer 1, fp32 write-out for layer 2. No transposes
  or weight matmuls in the inner loop.
"""
import sys
sys.path.insert(0, "/opt/trn_rl_repo")
import numpy as np
import ml_dtypes

import concourse.bass as bass
import concourse.bacc as bacc
import concourse.mybir as mybir
import concourse.tile as tile
from concourse.bass_utils import run_bass_kernel_spmd
from concourse.masks import make_identity

BF16 = ml_dtypes.bfloat16

N_NODES = 100000
N_EDGES = 1600000
F_IN = 128
F_OUT = 64
P = 8                  # cores
NREAL = 12500          # real dsts per core
NL = 12544             # padded dsts per core (= 98 * 128)
BLK = 128              # dsts per block
NB = NL // BLK         # 98 blocks
NG = P * NL            # 100352 padded global rows
CHUNK = 25088          # rows per gather chunk (4 * 25088 = NG, int16-safe)
NCHUNK = NG // CHUNK   # 4
GROUP = 7              # blocks per gather group
NGROUP = NB // GROUP   # 14
SENT = 999.0           # sentinel seg id (exact in bf16, != any iota value)

# gather call shape: None => one call per (group, chunk); else max cols/call
SINGLE_PACKET = False
CALL_COLS = None


def _wrap16(flat_idx):
    """dma_gather index layout: position j -> [j%16, j//16], replicated x8."""
    w = flat_idx.reshape(-1, 16).T.copy()
    return np.tile(w, (8, 1))


def _preprocess(edge_index):
    """Host-side graph layout (structure only, no feature math)."""
    src = np.asarray(edge_index[0], dtype=np.int64)
    dst = np.asarray(edge_index[1], dtype=np.int64)
    dcore = dst // NREAL
    dslot = dst - dcore * NREAL
    score = src // NREAL
    g_src = score * NL + (src - score * NREAL)

    blk = dslot // BLK
    m_in_blk = dslot % BLK
    chunk = g_src // CHUNK
    loc = g_src - chunk * CHUNK          # 0..25087, int16-safe

    # per-core 1/cnt for the mean
    rcnt_all = []
    for k in range(P):
        sel = dcore == k
        cnt = np.bincount(dslot[sel], minlength=NL).astype(np.float32)
        rcnt = 1.0 / np.maximum(cnt, 1.0)
        rcnt_all.append(rcnt.reshape(NB, BLK).T.copy())  # [128, NB]

    # group edges by (core, block, chunk); count -> uniform cols
    key = (dcore * NB + blk) * NCHUNK + chunk
    order = np.argsort(key, kind="stable")
    key_o = key[order]
    loc_o = loc[order]
    m_o = m_in_blk[order]
    counts = np.bincount(key_o, minlength=P * NB * NCHUNK).reshape(P, NB, NCHUNK)
    starts_flat = np.concatenate([[0], np.cumsum(counts.reshape(-1))])
    ncols_u = np.ceil(counts.max(axis=0) / 128).astype(np.int64)  # [NB, NCHUNK]
    ncols_u = np.maximum(ncols_u, 1)

    # position stream: for group: for chunk: for block in group: segment
    # seg_col_off[b][c] = starting column of that segment (global)
    seg_col_off = np.zeros((NB, NCHUNK), dtype=np.int64)
    group_col_off = []           # starting column of each group
    calls = []                   # (chunk, global_start_col, ncols)
    col = 0
    for g in range(NGROUP):
        group_col_off.append(col)
        for c in range(NCHUNK):
            run_start = col
            for b in range(g * GROUP, (g + 1) * GROUP):
                seg_col_off[b, c] = col
                col += int(ncols_u[b, c])
            run_cols = col - run_start
            done = 0
            cap = run_cols if CALL_COLS is None else CALL_COLS
            while done < run_cols:
                piece = min(run_cols - done, cap)
                calls.append((c, run_start + done, piece))
                done += piece
    total_cols = col
    group_cols = [group_col_off[g + 1] - group_col_off[g]
                  for g in range(NGROUP - 1)] + [total_cols - group_col_off[-1]]

    # per-core idx / seg streams
    idx_cores = []
    seg_cores = []
    for k in range(P):
        idx_flat = np.zeros(total_cols * 128, dtype=np.int16)
        seg_flat = np.full(total_cols * 128, SENT, dtype=np.float32)
        for b in range(NB):
            for c in range(NCHUNK):
                i = (k * NB + b) * NCHUNK + c
                s, e = starts_flat[i], starts_flat[i + 1]
                n = e - s
                st = seg_col_off[b, c] * 128
                idx_flat[st:st + n] = loc_o[s:e]
                seg_flat[st:st + n] = m_o[s:e]
        idx_cores.append(_wrap16(idx_flat))                    # [128, total_cols*8]
        seg_cores.append(
            seg_flat.reshape(total_cols, 128).T.astype(BF16).copy())  # [128, TC]

    ncmax = int(ncols_u.max())
    return dict(calls=calls, ncols_u=ncols_u, seg_col_off=seg_col_off,
                group_col_off=group_col_off, group_cols=group_cols,
                total_cols=total_cols, ncmax=ncmax,
                idx_cores=idx_cores, seg_cores=seg_cores, rcnt=rcnt_all)


def _build(meta, rep=1):
    calls = meta["calls"]
    ncols_u = meta["ncols_u"]
    seg_col_off = meta["seg_col_off"]
    group_col_off = meta["group_col_off"]
    TC = meta["total_cols"]
    GC_max = max(meta["group_cols"])
    ncmax = meta["ncmax"]

    nc = bacc.Bacc("TRN2", target_bir_lowering=False, debug=False,
                   num_devices=P, num_swdge_queues=4)
    dt = mybir.dt
    xT_d = nc.dram_tensor("xT", [F_IN, NL], dt.bfloat16, kind="ExternalInput")
    idx_d = nc.dram_tensor("idx", [128, TC * 8], dt.int16, kind="ExternalInput")
    seg_d = nc.dram_tensor("seg", [128, TC], dt.bfloat16, kind="ExternalInput")
    rcnt_d = nc.dram_tensor("rcnt", [128, NB], dt.float32, kind="ExternalInput")
    iota_d = nc.dram_tensor("iota", [128, ncmax * 128], dt.bfloat16,
                            kind="ExternalInput")
    wl1_d = nc.dram_tensor("W_l1", [F_IN, F_OUT], dt.bfloat16, kind="ExternalInput")
    wr1_d = nc.dram_tensor("W_r1", [F_IN, F_OUT], dt.bfloat16, kind="ExternalInput")
    b1_d = nc.dram_tensor("b1", [1, F_OUT], dt.bfloat16, kind="ExternalInput")
    wl2_d = nc.dram_tensor("W_l2", [F_OUT, F_OUT], dt.bfloat16, kind="ExternalInput")
    wr2_d = nc.dram_tensor("W_r2", [F_OUT, F_OUT], dt.bfloat16, kind="ExternalInput")
    b2_d = nc.dram_tensor("b2", [1, F_OUT], dt.bfloat16, kind="ExternalInput")
    out_d = nc.dram_tensor("out", [NL, F_OUT], dt.float32, kind="ExternalOutput")

    ytab1_shard = nc.dram_tensor("ytab1_shard", [NL, F_IN], dt.bfloat16)
    ytab1 = nc.dram_tensor("ytab1", [NG, F_IN], dt.bfloat16, addr_space="Shared")
    ytab2_shard = nc.dram_tensor("ytab2_shard", [NL, F_IN], dt.bfloat16)
    ytab2 = nc.dram_tensor("ytab2", [NG, F_IN], dt.bfloat16, addr_space="Shared")

    ident = mybir.ActivationFunctionType
    alu = mybir.AluOpType

    with tile.TileContext(nc) as tc:
        with (
            tc.tile_pool(name="const", bufs=1) as constp,
            tc.tile_pool(name="persist", bufs=1) as persistp,
            tc.tile_pool(name="xp", bufs=3) as xp,
            tc.tile_pool(name="stagep", bufs=2) as stagep,
            tc.tile_pool(name="indp", bufs=4) as indp,
            tc.tile_pool(name="op", bufs=4) as op,
            tc.tile_pool(name="ps_acc", bufs=2, space="PSUM") as ps_acc,
            tc.tile_pool(name="ps_y", bufs=2, space="PSUM") as ps_y,
            tc.tile_pool(name="ps_t", bufs=2, space="PSUM") as ps_t,
        ):
            idx_t = constp.tile([128, TC * 8], dt.int16)
            nc.sync.dma_start(idx_t[:], idx_d[:])
            seg_t = constp.tile([128, TC], dt.bfloat16)
            nc.sync.dma_start(seg_t[:], seg_d[:])
            rcnt_t = constp.tile([128, NB], dt.float32)
            nc.sync.dma_start(rcnt_t[:], rcnt_d[:])
            iota_t = constp.tile([128, ncmax * 128], dt.bfloat16)
            nc.sync.dma_start(iota_t[:], iota_d[:])
            wl1_t = constp.tile([F_IN, F_OUT], dt.bfloat16)
            nc.sync.dma_start(wl1_t[:], wl1_d[:])
            wr1_t = constp.tile([F_IN, F_OUT], dt.bfloat16)
            nc.sync.dma_start(wr1_t[:], wr1_d[:])
            wl2_t = constp.tile([F_OUT, F_OUT], dt.bfloat16)
            nc.sync.dma_start(wl2_t[:], wl2_d[:])
            wr2_t = constp.tile([F_OUT, F_OUT], dt.bfloat16)
            nc.sync.dma_start(wr2_t[:], wr2_d[:])
            b1_t = constp.tile([1, F_OUT], dt.bfloat16)
            nc.sync.dma_start(b1_t[:], b1_d[:])
            b2_t = constp.tile([1, F_OUT], dt.bfloat16)
            nc.sync.dma_start(b2_t[:], b2_d[:])
            ones_t = constp.tile([1, 128], dt.bfloat16)
            nc.vector.memset(ones_t[:], 1.0)
            id_t = constp.tile([128, 128], dt.bfloat16)
            make_identity(nc, id_t[:])
            # self-term buffer: y1r during L1, overwritten by y2r per block
            yr_t = persistp.tile([128, NB * F_OUT], dt.float32)

            qn = [0]

            def gather_group(g, table, stage):
                c0 = group_col_off[g]
                for (c, start_col, piece) in calls:
                    if not (c0 <= start_col < c0 + meta["group_cols"][g]):
                        continue
                    w0 = start_col - c0
                    nc.gpsimd.dma_gather(
                        out_ap=stage[:, w0 * F_IN:(w0 + piece) * F_IN]
                            .rearrange("p (c f) -> p c f", f=F_IN),
                        in_ap=table[c * CHUNK:(c + 1) * CHUNK, :],
                        idxs_ap=idx_t[:, start_col * 8:(start_col + piece) * 8],
                        num_idxs=piece * 128, num_idxs_reg=piece * 128,
                        elem_size=F_IN, single_packet=SINGLE_PACKET,
                        queue_num=qn[0] % 4)
                    qn[0] += 1

            def aggregate_block(b, g, stage):
                """acc[128 dst, 64] = sum of staged messages for block b."""
                c0 = group_col_off[g]
                acc = ps_acc.tile([BLK, F_OUT], dt.float32, tag="acc")
                nsegs = [int(ncols_u[b, c]) for c in range(NCHUNK)]
                total = sum(nsegs)
                done = 0
                for c in range(NCHUNK):
                    ncol = nsegs[c]
                    if ncol == 0:
                        continue
                    s0 = seg_col_off[b, c]
                    w0 = s0 - c0
                    ind = indp.tile([128, ncmax * 128], dt.bfloat16, tag="ind")
                    nc.vector.scalar_tensor_tensor(
                        out=ind[:, :ncol * 128]
                            .rearrange("p (c f) -> p c f", f=128),
                        in0=iota_t[:, :ncol * 128]
                            .rearrange("p (c f) -> p c f", f=128),
                        scalar=0.0,
                        in1=seg_t[:, s0:s0 + ncol].unsqueeze(2)
                            .broadcast_to([128, ncol, 128]),
                        op0=alu.add, op1=alu.is_equal)
                    for j in range(ncol):
                        nc.tensor.matmul(
                            acc[:],
                            lhsT=ind[:, j * 128:(j + 1) * 128],
                            rhs=stage[:, (w0 + j) * F_IN:(w0 + j) * F_IN + F_OUT],
                            start=(done == 0), stop=(done == total - 1))
                        done += 1
                return acc

            def shard_table_tile(b, lhsT_ap, wl, wr, bias, tab_dram):
                """y_l tile -> bf16 table rows; y_r tile (+bias) -> yr_t."""
                psl = ps_y.tile([128, F_OUT], dt.float32, tag="psl")
                nc.tensor.matmul(psl[:], lhsT=lhsT_ap, rhs=wl[:],
                                 start=True, stop=True)
                ya = op.tile([128, F_OUT], dt.bfloat16, tag="ya")
                nc.scalar.activation(out=ya[:], in_=psl[:], func=ident.Copy)
                nc.sync.dma_start(tab_dram[b * 128:(b + 1) * 128, :F_OUT], ya[:])
                psr = ps_y.tile([128, F_OUT], dt.float32, tag="psr")
                nc.tensor.matmul(psr[:], lhsT=lhsT_ap, rhs=wr[:],
                                 start=True, stop=False)
                nc.tensor.matmul(psr[:], lhsT=ones_t[:1, :128], rhs=bias[:],
                                 start=False, stop=True)
                nc.vector.tensor_copy(yr_t[:, b * F_OUT:(b + 1) * F_OUT], psr[:])

            for _r in range(rep):
                # ---- phase A: y1 tables from x shard ----
                for b in range(NB):
                    xt = xp.tile([F_IN, 128], dt.bfloat16, tag="xt")
                    nc.sync.dma_start(xt[:], xT_d[:, b * 128:(b + 1) * 128])
                    shard_table_tile(b, xt[:], wl1_t, wr1_t, b1_t, ytab1_shard)
                nc.gpsimd.collective_compute(
                    "AllGather", alu.bypass,
                    replica_groups=[list(range(P))],
                    ins=[ytab1_shard[:]], outs=[ytab1[:]])

                # ---- layer 1 aggregation + h1/y2 prep ----
                for g in range(NGROUP):
                    stage = stagep.tile([128, GC_max * F_IN], dt.bfloat16,
                                        tag="stage")
                    gather_group(g, ytab1, stage)
                    for b in range(g * GROUP, (g + 1) * GROUP):
                        acc = aggregate_block(b, g, stage)
                        t1 = op.tile([128, F_OUT], dt.float32, tag="t1")
                        nc.vector.scalar_tensor_tensor(
                            out=t1[:], in0=acc[:], scalar=rcnt_t[:, b:b + 1],
                            in1=yr_t[:, b * F_OUT:(b + 1) * F_OUT],
                            op0=alu.mult, op1=alu.add)
                        h1b = op.tile([128, F_OUT], dt.bfloat16, tag="h1b")
                        nc.vector.tensor_scalar_max(h1b[:], t1[:], 0.0)
                        # transpose h1 block for the y2 matmuls
                        tp = ps_t.tile([F_OUT, 128], dt.bfloat16, tag="tp")
                        nc.tensor.transpose(out=tp[:], in_=h1b[:],
                                            identity=id_t[:])
                        h1t = op.tile([F_OUT, 128], dt.bfloat16, tag="h1t")
                        nc.scalar.activation(out=h1t[:], in_=tp[:],
                                             func=ident.Copy)
                        shard_table_tile(b, h1t[:], wl2_t, wr2_t, b2_t,
                                         ytab2_shard)
                nc.gpsimd.collective_compute(
                    "AllGather", alu.bypass,
                    replica_groups=[list(range(P))],
                    ins=[ytab2_shard[:]], outs=[ytab2[:]])

                # ---- layer 2 aggregation ----
                for g in range(NGROUP):
                    stage = stagep.tile([128, GC_max * F_IN], dt.bfloat16,
                                        tag="stage")
                    gather_group(g, ytab2, stage)
                    for b in range(g * GROUP, (g + 1) * GROUP):
                        acc = aggregate_block(b, g, stage)
                        ot = op.tile([128, F_OUT], dt.float32, tag="ot")
                        nc.vector.scalar_tensor_tensor(
                            out=ot[:], in0=acc[:], scalar=rcnt_t[:, b:b + 1],
                            in1=yr_t[:, b * F_OUT:(b + 1) * F_OUT],
                            op0=alu.mult, op1=alu.add)
                        nc.sync.dma_start(out_d[b * 128:(b + 1) * 128, :], ot[:])

    nc.finalize()
    return nc


def make_in_maps(x, W_l1, W_r1, b1, W_l2, W_r2, b2, meta):
    x = np.asarray(x, dtype=np.float32)
    xT_all = np.zeros((P, F_IN, NL), dtype=BF16)
    for k in range(P):
        xs = np.zeros((NL, F_IN), np.float32)
        xs[:NREAL] = x[k * NREAL:(k + 1) * NREAL]
        xT_all[k] = xs.T.astype(BF16)
    ncmax = meta["ncmax"]
    iota = np.tile(np.arange(128, dtype=np.float32), ncmax)
    iota = np.broadcast_to(iota.astype(BF16), (128, ncmax * 128)).copy()
    in_maps = []
    for k in range(P):
        in_maps.append({
            "xT": xT_all[k],
            "idx": meta["idx_cores"][k],
            "seg": meta["seg_cores"][k],
            "rcnt": meta["rcnt"][k],
            "iota": iota,
            "W_l1": np.asarray(W_l1, np.float32).astype(BF16),
            "W_r1": np.asarray(W_r1, np.float32).astype(BF16),
            "b1": np.asarray(b1, np.float32).reshape(1, F_OUT).astype(BF16),
            "W_l2": np.asarray(W_l2, np.float32).astype(BF16),
            "W_r2": np.asarray(W_r2, np.float32).astype(BF16),
            "b2": np.asarray(b2, np.float32).reshape(1, F_OUT).astype(BF16),
        })
    return in_maps


def kernel(x, edge_index, W_l1, W_r1, b1, W_l2, W_r2, b2):
    meta = _preprocess(np.asarray(edge_index))
    in_maps = make_in_maps(x, W_l1, W_r1, b1, W_l2, W_r2, b2, meta)
    nc = _build(meta)
    res = run_bass_kernel_spmd(nc, in_maps, core_ids=list(range(P)))
    out = np.concatenate(
        [res.results[k]["out"][:NREAL] for k in range(P)], axis=0)
    return out.astype(np.float32)


if __name__ == "__main__":
    rng = np.random.default_rng(0)
    x = rng.normal(size=(N_NODES, F_IN)).astype(np.float32)
    ei = rng.integers(0, N_NODES, size=(2, N_EDGES)).astype(np.int64)
    wl1 = rng.normal(size=(F_IN, F_OUT)).astype(np.float32) / np.sqrt(F_IN)
    wr1 = rng.normal(size=(F_IN, F_OUT)).astype(np.float32) / np.sqrt(F_IN)
    wl2 = rng.normal(size=(F_OUT, F_OUT)).astype(np.float32) / np.sqrt(F_OUT)
    wr2 = rng.normal(size=(F_OUT, F_OUT)).astype(np.float32) / np.sqrt(F_OUT)
    b1 = np.zeros(F_OUT, np.float32)
    b2 = np.zeros(F_OUT, np.float32)
    out = kernel(x, ei, wl1, wr1, b1, wl2, wr2, b2)
    print("out", out.shape, out.dtype, float(np.abs(out).mean()))
